# revision 24
# baseline (speedup 1.0000x reference)
"""Multi-head causal attention (B=2, T=2048, C=1024, H=16) on 8 Trainium2
NeuronCores, tensor-parallel over heads (2 heads per core).

Layout strategy (everything column-major on device, i.e. feature = SBUF
partition dim, token = free dim):
  - host feeds xT [C, B*T]; per-core w_qkv column slices / w_out row slice.
  - phase 1: QT/KT/VT [128, 4096] = w_c.T @ xT   (accumulate over 8 k-tiles)
  - phase 1.5: PE-transpose V into natural [token, dim] layout, augmented
    with a ones column (row-sums of attention weights come free in AV).
  - phase 2/3 (flash-style, causal tiles skipped): per (batch, q-chunk 512):
      ST[k,q] = KT_tile.T @ QT_chunk  -> +mask on diagonal tiles
      PT = exp(ST/8) (ScalarE, PSUM->SBUF)
      OT_aug[65, q] += Vaug_tile.T @ PT  (row 64 = softmax denominator)
    normalize via reciprocal + ones outer-product broadcast, then
      yT[m-tile, q-chunk] = w_out_tile.T @ OT  -> DMA out.
  - host: sum 8 partial yT, transpose, add b_out.

All matmuls run in float32r (full PE rate at N=512); tiles that feed a
matmul are produced as float32r (walrus requires producer-side rounding).
"""

import os
import sys

for _p in ("/opt/trn_rl_repo", "/root/.axon_site/_ro/trn_rl_repo"):
    if os.path.isdir(_p) and _p not in sys.path:
        sys.path.insert(0, _p)

import numpy as np

import concourse.bacc as bacc
import concourse.bass as bass
import concourse.mybir as mybir
import concourse.tile as tile
from concourse.bass_utils import run_bass_kernel_spmd
from concourse.masks import make_identity

B, T, C, H, D = 2, 2048, 1024, 16, 64
NCORES = 8
BT = B * T                      # 4096 flattened tokens
TC = 512                        # token chunk (matmul free dim)
NTC = BT // TC                  # 8 token chunks
FP = mybir.dt.float32
FPR = mybir.dt.float32r
ACT = mybir.ActivationFunctionType
NEG = -1.0e9

LAST_RESULTS = None             # stashed BassKernelResults for test harness


def build_nc():
    nc = bacc.Bacc(None, target_bir_lowering=False, debug=False)

    xt = nc.declare_dram_parameter("xt", [C, BT], FP, isOutput=False)
    wc = nc.declare_dram_parameter("wc", [C, 384], FP, isOutput=False)
    wout = nc.declare_dram_parameter("wout", [128, C], FP, isOutput=False)
    bqkv = nc.declare_dram_parameter("bqkv", [128, 3], FP, isOutput=False)
    masks = nc.declare_dram_parameter("masks", [512, 512], FP, isOutput=False)
    ones = nc.declare_dram_parameter("ones", [128, 64], FP, isOutput=False)
    yt = nc.declare_dram_parameter("yt", [C, BT], FP, isOutput=True)

    with tile.TileContext(nc) as tc:
        with (
            tc.tile_pool(name="const", bufs=1) as cpool,
            tc.tile_pool(name="big", bufs=1) as bigpool,
            tc.tile_pool(name="sb", bufs=2) as sbpool,
            tc.tile_pool(name="ps", bufs=2, space="PSUM") as pspool,
        ):
            # ---- constants ----
            wc_sb = cpool.tile([128, 8 * 384], FPR)     # [cin, k*384 + g*128 + col]
            nc.sync.dma_start(
                out=wc_sb[:].rearrange("b (a c) -> b a c", a=8),
                in_=wc.bitcast(FPR).rearrange("(a b) c -> b a c", a=8),
            )
            # w_out split per local head so the out-projection can contract
            # each head from partition base 0 (fp32r matmul dst/base rules)
            wout_sbs = (cpool.tile([64, C], FPR, name="wout0"),
                        cpool.tile([64, C], FPR, name="wout1"))
            nc.sync.dma_start(out=wout_sbs[0][:], in_=wout[0:64, :].bitcast(FPR))
            nc.sync.dma_start(out=wout_sbs[1][:], in_=wout[64:128, :].bitcast(FPR))
            bq_sb = cpool.tile([128, 3], FP)
            nc.sync.dma_start(out=bq_sb[:], in_=bqkv[:, :])
            masks_sb = cpool.tile([128, 4 * 512], FP)
            nc.sync.dma_start(
                out=masks_sb[:].rearrange("b (a c) -> b a c", a=4),
                in_=masks.rearrange("(a b) c -> b a c", a=4),
            )
            ones_sb = cpool.tile([128, 64], FPR)
            nc.sync.dma_start(out=ones_sb[:], in_=ones.bitcast(FPR)[:, :])
            ident = cpool.tile([128, 128], FP)
            make_identity(nc, ident)

            # ---- persistent intermediates ----
            QT = bigpool.tile([128, BT], FPR)
            KT = bigpool.tile([128, BT], FPR)
            VT = bigpool.tile([128, BT], FP)
            # V in [token, dim] layout, 65 cols per 128-token block
            # (col 64 stays 1.0 -> attention row-sums)
            vaugs = (bigpool.tile([128, 32 * 65], FPR, name="vaug0"),
                     bigpool.tile([128, 32 * 65], FPR, name="vaug1"))
            for vg in vaugs:
                nc.sync.dma_start(
                    out=vg[:].rearrange("p (j c) -> p j c", c=65)[:, :, 64:65],
                    in_=ones.bitcast(FPR).rearrange("p (j c) -> p j c", c=1)[:, 0:32, :],
                )

            qkvT = (QT, KT, VT)

            for tcx in range(NTC):
                b, qc = divmod(tcx, 4)
                t0 = tcx * TC

                # ---- phase 1: QKV projection for this token chunk ----
                xts = []
                for k in range(8):
                    xtile = sbpool.tile([128, TC], FPR, tag="xt", bufs=10)
                    nc.sync.dma_start(
                        out=xtile[:],
                        in_=xt[k * 128:(k + 1) * 128, t0:t0 + TC].bitcast(FPR),
                    )
                    xts.append(xtile)
                for g in range(3):
                    ps = pspool.tile([128, TC], FP, tag="qkv", bufs=2)
                    for k in range(8):
                        nc.tensor.matmul(
                            ps[:],
                            wc_sb[:, k * 384 + g * 128:k * 384 + (g + 1) * 128],
                            xts[k][:],
                            start=(k == 0),
                            stop=(k == 7),
                        )
                    nc.scalar.activation(
                        qkvT[g][:, t0:t0 + TC], ps[:], ACT.Identity,
                        bias=bq_sb[:, g:g + 1],
                    )

                # ---- phase 1.5: transpose this chunk's V into vaugs ----
                for j in range(4):
                    jj = tcx * 4 + j
                    tp = pspool.tile([128, 128], FP, tag="y", bufs=2)
                    nc.tensor.transpose(
                        tp[:], VT[:, jj * 128:(jj + 1) * 128], ident[:]
                    )
                    nc.scalar.copy(vaugs[0][:, jj * 65:jj * 65 + 64], tp[:, 0:64])
                    nc.scalar.copy(vaugs[1][:, jj * 65:jj * 65 + 64], tp[:, 64:128])

                # ---- phase 2/3: causal attention for (b, qc) ----
                n_kt = 4 * (qc + 1)
                otps = [
                    pspool.tile([65, TC], FP, tag="av", bufs=2, name=f"otp{_h}")
                    for _h in range(2)
                ]
                for kt in range(n_kt):
                    kg = b * 16 + kt
                    for h in range(2):
                        sp = pspool.tile([128, TC], FP, tag="s", bufs=2)
                        nc.tensor.matmul(
                            sp[:],
                            KT[h * 64:(h + 1) * 64, kg * 128:(kg + 1) * 128],
                            QT[h * 64:(h + 1) * 64, t0:t0 + TC],
                            start=True, stop=True,
                        )
                        if kt >= 4 * qc:
                            v = kt - 4 * qc
                            nc.vector.tensor_add(
                                sp[:], sp[:], masks_sb[:, v * 512:(v + 1) * 512]
                            )
                        pt = sbpool.tile([128, TC], FPR, tag="pt", bufs=3)
                        nc.scalar.activation(pt[:], sp[:], ACT.Exp, scale=0.125)
                        nc.tensor.matmul(
                            otps[h][:],
                            vaugs[h][:, kg * 65:kg * 65 + 65],
                            pt[:],
                            start=(kt == 0), stop=(kt == n_kt - 1),
                            skip_group_check=True,
                        )
                ots = []
                for h in range(2):
                    rc = sbpool.tile([1, TC], FPR, tag=f"rc{h}", bufs=2,
                                     name=f"rc{h}")
                    with nc.allow_low_precision(reason="softmax recip in f32r"):
                        nc.vector.reciprocal(rc[:], otps[h][64:65, :])
                    bch = pspool.tile([64, TC], FP, tag="s", bufs=2,
                                      name=f"bc{h}")
                    nc.tensor.matmul(bch[:], ones_sb[0:1, :], rc[:],
                                     start=True, stop=True)
                    bcs = sbpool.tile([64, TC], FP, tag=f"bcs{h}", bufs=2,
                                      name=f"bcs{h}")
                    nc.scalar.copy(bcs[:], bch[:])
                    oth = sbpool.tile([64, TC], FPR, tag=f"ot{h}", bufs=2,
                                      name=f"ot{h}")
                    nc.vector.tensor_mul(oth[:], otps[h][0:64, :], bcs[:])
                    ots.append(oth)

                # ---- phase 4: output projection (contract heads) ----
                for m in range(8):
                    yp = pspool.tile([128, TC], FP, tag="y", bufs=2)
                    for h in range(2):
                        nc.tensor.matmul(
                            yp[:],
                            wout_sbs[h][:, m * 128:(m + 1) * 128],
                            ots[h][:],
                            start=(h == 0), stop=(h == 1),
                        )
                    ys = sbpool.tile([128, TC], FP, tag="ys", bufs=3)
                    nc.vector.tensor_copy(ys[:], yp[:])
                    nc.sync.dma_start(
                        out=yt[m * 128:(m + 1) * 128, t0:t0 + TC], in_=ys[:]
                    )
    nc.compile()
    return nc


def make_in_maps(x, w_qkv, b_qkv):
    x = np.ascontiguousarray(np.asarray(x, np.float32).reshape(BT, C))
    xT = np.ascontiguousarray(x.T)
    w_qkv = np.asarray(w_qkv, np.float32)
    b_qkv = np.asarray(b_qkv, np.float32)

    mask = np.empty((512, 512), np.float32)
    for v in range(4):
        kk = np.arange(128)[:, None] + 128 * v
        qq = np.arange(512)[None, :]
        mask[v * 128:(v + 1) * 128] = np.where(kk <= qq, 0.0, NEG)

    in_maps = []
    for c in range(NCORES):
        sl = slice(c * 128, (c + 1) * 128)
        wc = np.concatenate(
            [w_qkv[:, sl], w_qkv[:, 1024:][:, sl], w_qkv[:, 2048:][:, sl]], axis=1
        )
        bq = np.stack(
            [b_qkv[sl], b_qkv[1024:][sl], b_qkv[2048:][sl]], axis=1
        )
        in_maps.append({
            "xt": xT,
            "wc": np.ascontiguousarray(wc),
            "wout": None,  # filled by caller (needs w_out)
            "bqkv": np.ascontiguousarray(bq),
            "masks": mask,
            "ones": np.ones((128, 64), np.float32),
        })
    return in_maps


_NC_CACHE = None


def kernel(x, w_qkv, b_qkv, w_out, b_out):
    global _NC_CACHE, LAST_RESULTS
    if _NC_CACHE is None:
        _NC_CACHE = build_nc()
    nc = _NC_CACHE

    w_out = np.asarray(w_out, np.float32)
    in_maps = make_in_maps(x, w_qkv, b_qkv)
    for c in range(NCORES):
        in_maps[c]["wout"] = np.ascontiguousarray(w_out[c * 128:(c + 1) * 128, :])

    res = run_bass_kernel_spmd(
        nc, in_maps, list(range(NCORES)),
        trace=bool(os.environ.get("BASS_TRACE")),
    )
    LAST_RESULTS = res

    acc = np.zeros((C, BT), np.float64)
    for out_map in res.results:
        acc += out_map["yt"].astype(np.float64)
    y = acc.T.astype(np.float32) + np.asarray(b_out, np.float32)[None, :]
    return y.reshape(B, T, C)
